# revision 65
# baseline (speedup 1.0000x reference)
"""Sliding-window (chunked) multi-head attention for Trainium2, 8-core SPMD.

Problem: B=1, S=8192, E=512, H=8 heads, Dh=64, window=1024 (half=512).
Reference math per window i (size 1024): keys span [i-512, i+1536).

Sharding: core c owns query window [1024c, 1024c+1024); it receives
x^T for the halo'd key range [1024c-512, 1024c+1536) (zero-padded at
the sequence edges) and computes q/k/v projections locally, windowed
softmax(q k^T / 8) v, and the output projection.  All compute layouts
are transposed ([E, seq]) so every matmul contracts over partitions;
the softmax denominator comes from a ones-augmented v (65th column).
bv is folded into an adjusted output-projection bias on the host
(attn rows sum to 1), so v needs no bias add on-chip.

Pipeline layout: per head-pair and query-half, the attention loop is
software-pipelined one batch deep -- two key-tiles of scores issue
back-to-back (the two heads' 64-row matmuls run as concurrent PE
tiles), then the previous batch's four attn*v matmuls run as a block.
Batching this way halves the PE row-config switches (64-row score
tiles vs 128-row av), which the trace showed cost ~120ns per switch.
exp runs on vector (custom cubic^4 DVE op) for head 0 of the pair and
on the scalar ACT engine for head 1, fully hidden under the tensor
stream.  Normalization is per query-half: vector/scalar evacuate the
AV PSUM (the ones-column denominator rides along), the dens rows
gather via SBUF-SBUF DMA to partition 0 (custom DVE ops misread
nonzero partition offsets), vector computes 1/dens, a DRAM-roundtrip
DMA broadcasts it across 64 partitions, and gpsimd applies the scale.
The y projection accumulates ke=0..2 for all 8 output tiles during
the last normalization chain; only the ke=3 matmuls wait on it.

Outputs are y^T shards [512, 1024] per core; the host transposes and
concatenates.
"""

import numpy as np
import ml_dtypes

import concourse.bass as bass
import concourse.tile as tile
from concourse import bacc, mybir
from concourse import bass_utils
from concourse.bass import ts

# ---- problem constants (hardcoded per contract) ----
S = 8192
E = 512
H = 8
DH = 64
NCORES = 8
SQ = 1024          # queries per core
SK = 2048          # halo'd keys per core
HALF = 512
SCALE = 0.125      # 1/sqrt(64)

F32 = mybir.dt.float32
BF16 = mybir.dt.bfloat16
FP16 = mybir.dt.float16
FP8 = mybir.dt.float8e4

# ---- custom DVE op: exp(u/8) ~= (1 + c1 u + c2 u^2 + c3 u^3)^4 ----
# Fitted (Lawson minimax) on |u/8| <= 1.6; max rel err 7.2e-4.
_EC1 = 0.03126080224663743
_EC2 = 0.000493647595612354
_EC3 = 5.0261583805949835e-06


def _register_exp_op():
    from concourse import dve_ops as dops
    from concourse.dve_spec import Spec, Src0, One, C0, C1, C2, sq, lower
    from concourse.dve_uop import DveOpSpec

    name = "EXP4_ANT"
    for op in dops.OPS:
        if op.name == name:
            return op
    body = sq(sq(((C2 * Src0 + C1) * Src0 + C0) * Src0 + One))
    spec = Spec(body=body)
    shas = {}
    for ver in ("v3", "v4"):
        uops = lower(spec, ver=ver)
        shas[ver] = DveOpSpec(name=name, opcode=0, uops=uops, rd1_en=False).sha(ver)
    op = dops.DveOp(name, spec, subdim=False, uops_sha=shas)
    dops.OPS.append(op)
    dops.CUSTOM_DVE_SPECS[name] = spec
    dops._SUB_OPCODE_FOR_NAME[name] = dops._CUSTOM_DVE_ROW_BASE + len(dops.OPS) - 1
    assert max(dops._SUB_OPCODE_FOR_NAME.values()) < 0x20
    return op


def _build():
    """Build + compile the per-core Bass program (SPMD: same NEFF, 8 cores)."""
    exp_op = _register_exp_op()

    nc = bacc.Bacc("TRN2", target_bir_lowering=False, debug=False)

    xT_d = nc.dram_tensor("xT", [E, SK], FP16, kind="ExternalInput")
    W_d = {
        n: nc.dram_tensor(n, [128, E // 128, E], FP16, kind="ExternalInput")
        for n in ("Wq", "Wk", "Wv", "Wo")
    }
    bq_d = nc.dram_tensor("bq", [E], F32, kind="ExternalInput")
    bk_d = nc.dram_tensor("bk", [E], F32, kind="ExternalInput")
    bo_d = nc.dram_tensor("bo_eff", [E], F32, kind="ExternalInput")
    mask_d = nc.dram_tensor("mask8", [128, H, SK // 128], FP16, kind="ExternalInput")
    yT_d = nc.dram_tensor("yT", [E, SQ], F32, kind="ExternalOutput")

    KT = 4   # E // 128 contraction tiles
    NKT = SK // 128  # 16 key tiles

    with tile.TileContext(nc) as tc:
        with (
            nc.allow_low_precision(reason="fp16/fp8 attention kernel"),
            tc.tile_pool(name="singles", bufs=1) as singles,
            tc.tile_pool(name="exps0", bufs=8) as exps0,
            tc.tile_pool(name="exps1", bufs=8) as exps1,
            tc.tile_pool(name="recips", bufs=2) as recips,
            tc.tile_pool(name="avus", bufs=2) as avus,
            tc.tile_pool(name="dscratch", bufs=2, space="DRAM") as dscratch,
            tc.tile_pool(name="bcs", bufs=2) as bcs,
            tc.tile_pool(name="ystage", bufs=3) as ystage,
        ):
            # ---- load everything ----
            W_sb = {}
            for n, d in W_d.items():
                W_sb[n] = singles.tile([128, KT, E], FP16, tag=f"w_{n}", name=f"w_{n}")
            xT_sb = singles.tile([128, KT, SK], FP16)
            # spread the startup-critical loads (x and Wq) round-robin over
            # three DMA queues so the first q-projection starts early
            qs = (nc.sync, nc.scalar, nc.gpsimd)
            for ke in range(KT):
                qs[ke % 3].dma_start(out=xT_sb[:, ke, :], in_=xT_d[ts(ke, 128), :])
                qs[(ke + 1) % 3].dma_start(
                    out=W_sb["Wq"][:, ke, :], in_=W_d["Wq"].ap()[:, ke, :]
                )
            for n, eng in (("Wk", nc.gpsimd), ("Wv", nc.sync), ("Wo", nc.scalar)):
                eng.dma_start(out=W_sb[n], in_=W_d[n].ap())
            bq_sb = singles.tile([128, KT], F32, tag="bq")
            nc.sync.dma_start(out=bq_sb, in_=bq_d.ap().rearrange("(t p) -> p t", p=128))
            bk_sb = singles.tile([128, KT], F32, tag="bk")
            nc.sync.dma_start(out=bk_sb, in_=bk_d.ap().rearrange("(t p) -> p t", p=128))
            bo_sb = singles.tile([128, KT], F32, tag="bo")
            nc.sync.dma_start(out=bo_sb, in_=bo_d.ap().rearrange("(t p) -> p t", p=128))

            # v with ones column (from mask: 0 for padded keys), head-major
            v_sb = singles.tile([128, H, NKT, DH + 1], FP16, tag="v")
            nc.sync.dma_start(out=v_sb[:, :, :, DH], in_=mask_d.ap())

            qT_sb = singles.tile([128, KT, SQ], FP16, tag="qT")
            kT_sb = singles.tile([128, KT, SK], FP16, tag="kT")
            outT_sb = singles.tile([128, KT, SQ], FP16, tag="outT")

            # ---- q/k/v projections ----
            # bias adds on vector, v evacuation on scalar (both idle here;
            # gpsimd cannot read PSUM).
            with tc.tile_pool(name="pproj", bufs=4, space="PSUM") as pproj:
                for th in range(KT):
                    for qc in range(2):
                        ps = pproj.tile([128, 512], F32, tag="pp")
                        for ke in range(KT):
                            nc.tensor.matmul(
                                ps,
                                W_sb["Wq"][:, ke, ts(th, 128)],
                                xT_sb[:, ke, HALF + qc * 512:HALF + (qc + 1) * 512],
                                start=(ke == 0), stop=(ke == KT - 1),
                            )
                        nc.vector.tensor_scalar_add(
                            out=qT_sb[:, th, ts(qc, 512)], in0=ps, scalar1=bq_sb[:, th:th + 1]
                        )
                for th in range(KT):
                    for kc in range(4):
                        ps = pproj.tile([128, 512], F32, tag="pp")
                        for ke in range(KT):
                            nc.tensor.matmul(
                                ps,
                                W_sb["Wk"][:, ke, ts(th, 128)],
                                xT_sb[:, ke, ts(kc, 512)],
                                start=(ke == 0), stop=(ke == KT - 1),
                            )
                        nc.vector.tensor_scalar_add(
                            out=kT_sb[:, th, ts(kc, 512)], in0=ps, scalar1=bk_sb[:, th:th + 1]
                        )
                for st in range(NKT):
                    ps = pproj.tile([128, 512], F32, tag="pp")
                    for ke in range(KT):
                        nc.tensor.matmul(
                            ps,
                            xT_sb[:, ke, ts(st, 128)],
                            W_sb["Wv"][:, ke, :],
                            start=(ke == 0), stop=(ke == KT - 1),
                        )
                    nc.scalar.activation(
                        out=v_sb[:, :, st, 0:DH],
                        in_=ps.rearrange("p (h d) -> p h d", h=H),
                        func=mybir.ActivationFunctionType.Copy,
                    )

            # ---- windowed attention ----
            # Per head-pair hp (heads 2hp on q/k rows 0:64, 2hp+1 on 64:128),
            # per query-half qc (512 queries), stream the 16 key tiles:
            # scores (two concurrent 64-row PE-tile matmuls) -> exp (vector
            # for head 0 of pair, scalar for head 1) into fp8 pair-tiles ->
            # fp8 DoubleRow attn*v accumulation every second key tile.
            # PSUM: 4 score bufs + 4 av accumulators = 8 banks.
            with (
                tc.tile_pool(name="pscore", bufs=6, space="PSUM") as pscore,
                tc.tile_pool(name="pav", bufs=2, space="PSUM") as pav,
            ):
                for hp in range(H // 2):
                    th = hp
                    for qc in range(2):
                        avps = {}
                        for i in range(2):
                            avps[(qc, i)] = pav.tile(
                                [DH + 1, 512], F32, tag="av", name=f"av{qc}{i}"
                            )
                        # software-pipelined 1 deep: issue scores(kt+1) before
                        # av(kt) so the av weight-loads hide under the score
                        # stream and the exps get a full iteration of slack
                        def scores_exp(kt):
                            s_tiles = []
                            for i in range(2):
                                r0 = 64 * i
                                s_ps = pscore.tile([128, 512], F32, tag="s", name=f"s{i}")
                                nc.tensor.matmul(
                                    s_ps,
                                    kT_sb[r0:r0 + 64, th, ts(kt, 128)],
                                    qT_sb[r0:r0 + 64, th, ts(qc, 512)],
                                    start=True, stop=True,
                                )
                                s_tiles.append(s_ps)
                            e0 = exps0.tile([128, 512], FP16, tag="e0", name="e0")
                            e1 = exps1.tile([128, 512], FP16, tag="e1", name="e1")
                            # custom DVE cubic^4 exp (coefficients fold in SCALE)
                            nc.vector._custom_dve(
                                exp_op, out=e0, in0=s_tiles[0],
                                s0=_EC1, s1=_EC2, imm2=_EC3,
                            )
                            nc.scalar.activation(
                                out=e1, in_=s_tiles[1],
                                func=mybir.ActivationFunctionType.Exp, scale=SCALE,
                            )
                            return e0, e1

                        def av(kt, es):
                            for i, e in enumerate(es):
                                h = 2 * hp + i
                                nc.tensor.matmul(
                                    avps[(qc, i)],
                                    v_sb[:, h, kt, :],
                                    e,
                                    start=(kt == 0), stop=(kt == NKT - 1),
                                )

                        # batch 3-4 key-tiles of scores, then their av
                        # matmuls back-to-back: amortizes the PE row-config
                        # switch (64-row score tiles vs 128-row av, ~110ns
                        # each way) over more matmuls.  Score-tile pool (6)
                        # bounds the batch size; exp pools hold two batches.
                        batches = [(0, 1, 2), (3, 4, 5), (6, 7, 8),
                                   (9, 10, 11), (12, 13, 14, 15)]
                        es = {}
                        prev = None
                        for b in batches:
                            for kt in b:
                                es[kt] = scores_exp(kt)
                            if prev is not None:
                                for kt in prev:
                                    av(kt, es.pop(kt))
                            prev = b
                        for kt in prev:
                            av(kt, es.pop(kt))

                        # ---- normalize this qc ----
                        # evacuate AV PSUM on vector/scalar (gpsimd can't
                        # read PSUM); the dens row (65th) rides along. Dens
                        # rows gather via SBUF-SBUF DMA, vector computes
                        # 1/dens, a DRAM-roundtrip DMA broadcasts it across
                        # partitions, and gpsimd applies the scale (all-SBUF).
                        avu = avus.tile([DH + 1, 2, 512], F32, tag="avu")
                        nc.vector.tensor_copy(out=avu[:, 0, :], in_=avps[(qc, 0)])
                        nc.scalar.activation(
                            out=avu[:, 1, :], in_=avps[(qc, 1)],
                            func=mybir.ActivationFunctionType.Copy,
                        )
                        dens = recips.tile([2, 512], F32, tag="dens")
                        for i in range(2):
                            nc.sync.dma_start(
                                out=dens[i:i + 1, :], in_=avu[DH:DH + 1, i, :]
                            )
                        recip_f = recips.tile([2, 512], F32, tag="rf")
                        nc.vector.reciprocal_approx_fast(out=recip_f, in_=dens)
                        r_dram = dscratch.tile([2, 512], F32, tag="rd")
                        nc.sync.dma_start(out=r_dram, in_=recip_f)
                        bc_sb = bcs.tile([DH, 2, 512], F32, tag="bc")
                        for i in range(2):
                            nc.sync.dma_start(
                                out=bc_sb[:, i, :],
                                in_=bass.AP(
                                    tensor=r_dram.tensor,
                                    offset=r_dram.offset + i * 512,
                                    ap=[[0, DH]] + [list(a) for a in r_dram.ap[1:]],
                                ),
                            )
                        for i in range(2):
                            r0 = 64 * i
                            # the last chain gates the y projection: vector
                            # (idle by then) beats gpsimd there
                            eng = (nc.vector if (hp == H // 2 - 1 and qc == 1)
                                   else nc.gpsimd)
                            eng.tensor_mul(
                                out=outT_sb[r0:r0 + 64, th, ts(qc, 512)],
                                in0=avu[0:DH, i, :],
                                in1=bc_sb[:, i, :],
                            )

                # ---- output projection ----
                # all 8 output tiles accumulate ke=0..2 first (their outT
                # slices are long done), overlapping the last head-pair's
                # normalization chain; only the ke=3 matmuls wait on it.
                # 6 PSUM tiles come from the score pool, 2 from the av pool.
                mq = [(m, qc) for m in range(KT) for qc in range(2)]
                pss = []
                for g, (m, qc) in enumerate(mq):
                    if g < 6:
                        ps = pscore.tile([128, 512], F32, tag="s", name="yps")
                    else:
                        ps = pav.tile([128, 512], F32, tag="av", name="ypsb")
                    pss.append(ps)
                    for ke in range(KT - 1):
                        nc.tensor.matmul(
                            ps,
                            W_sb["Wo"][:, ke, ts(m, 128)],
                            outT_sb[:, ke, ts(qc, 512)],
                            start=(ke == 0), stop=False,
                        )
                for ps, (m, qc) in zip(pss, mq):
                    nc.tensor.matmul(
                        ps,
                        W_sb["Wo"][:, KT - 1, ts(m, 128)],
                        outT_sb[:, KT - 1, ts(qc, 512)],
                        start=False, stop=True,
                    )
                    yst = ystage.tile([128, 512], F32, tag="y")
                    nc.vector.tensor_scalar_add(out=yst, in0=ps, scalar1=bo_sb[:, m:m + 1])
                    nc.sync.dma_start(out=yT_d[ts(m, 128), ts(qc, 512)], in_=yst)

    nc.compile()
    return nc


_NC_CACHE = []


def _get_nc():
    if not _NC_CACHE:
        _NC_CACHE.append(_build())
    return _NC_CACHE[0]


def _prep_inputs(x, Wq, bq, Wk, bk, Wv, bv, Wo, bo):
    x = np.asarray(x, np.float32)
    xT_full = np.ascontiguousarray(x[0].T)  # [E, S]
    bo_eff = (np.asarray(bo, np.float64)
              + np.asarray(bv, np.float64) @ np.asarray(Wo, np.float64)).astype(np.float32)
    def wprep(W):
        Wb = np.asarray(W, np.float32).astype(np.float16)
        return np.ascontiguousarray(Wb.reshape(4, 128, E).transpose(1, 0, 2))

    shared = {
        "Wq": wprep(Wq),
        "Wk": wprep(Wk),
        "Wv": wprep(Wv),
        "Wo": wprep(Wo),
        "bq": np.asarray(bq, np.float32),
        "bk": np.asarray(bk, np.float32),
        "bo_eff": bo_eff,
    }
    in_maps = []
    for c in range(NCORES):
        g0 = 1024 * c - HALF
        xT_halo = np.zeros((E, SK), np.float32)
        lo, hi = max(0, g0), min(S, g0 + SK)
        xT_halo[:, lo - g0:hi - g0] = xT_full[:, lo:hi]
        mask = np.zeros((SK, H), np.float32)
        mask[lo - g0:hi - g0, :] = 1.0
        mask = np.ascontiguousarray(mask.reshape(SK // 128, 128, H).transpose(1, 2, 0))
        m = dict(shared)
        m["xT"] = xT_halo.astype(np.float16)
        m["mask8"] = mask.astype(np.float16)
        in_maps.append(m)
    return in_maps


def run(inputs: dict, trace: bool = False):
    nc = _get_nc()
    in_maps = _prep_inputs(**inputs)
    res = bass_utils.run_bass_kernel_spmd(
        nc, in_maps, core_ids=list(range(NCORES)), trace=trace
    )
    y = np.concatenate([r["yT"].T for r in res.results], axis=0)[None]
    return np.ascontiguousarray(y.astype(np.float32)), res


def kernel(**inputs) -> np.ndarray:
    y, _ = run(inputs, trace=False)
    return y


# revision 68
# speedup vs baseline: 1.0111x; 1.0111x over previous
"""Sliding-window (chunked) multi-head attention for Trainium2, 8-core SPMD.

Problem: B=1, S=8192, E=512, H=8 heads, Dh=64, window=1024 (half=512).
Reference math per window i (size 1024): keys span [i-512, i+1536).

Sharding: core c owns query window [1024c, 1024c+1024); it receives
x^T for the halo'd key range [1024c-512, 1024c+1536) (zero-padded at
the sequence edges) and computes q/k/v projections locally, windowed
softmax(q k^T / 8) v, and the output projection.  All compute layouts
are transposed ([E, seq]) so every matmul contracts over partitions;
the softmax denominator comes from a ones-augmented v (65th column).
bv is folded into an adjusted output-projection bias on the host
(attn rows sum to 1), so v needs no bias add on-chip.

Pipeline layout: per head-pair and query-half, the attention loop is
software-pipelined one batch deep -- two key-tiles of scores issue
back-to-back (the two heads' 64-row matmuls run as concurrent PE
tiles), then the previous batch's four attn*v matmuls run as a block.
Batching this way halves the PE row-config switches (64-row score
tiles vs 128-row av), which the trace showed cost ~120ns per switch.
exp runs on vector (custom cubic^4 DVE op) for head 0 of the pair and
on the scalar ACT engine for head 1, fully hidden under the tensor
stream.  Normalization is per query-half: vector/scalar evacuate the
AV PSUM (the ones-column denominator rides along), the dens rows
gather via SBUF-SBUF DMA to partition 0 (custom DVE ops misread
nonzero partition offsets), vector computes 1/dens, a DRAM-roundtrip
DMA broadcasts it across 64 partitions, and gpsimd applies the scale.
The y projection accumulates ke=0..2 for all 8 output tiles during
the last normalization chain; only the ke=3 matmuls wait on it.

Outputs are y^T shards [512, 1024] per core; the host transposes and
concatenates.
"""

import numpy as np
import ml_dtypes

import concourse.bass as bass
import concourse.tile as tile
from concourse import bacc, mybir
from concourse import bass_utils
from concourse.bass import ts

# ---- problem constants (hardcoded per contract) ----
S = 8192
E = 512
H = 8
DH = 64
NCORES = 8
SQ = 1024          # queries per core
SK = 2048          # halo'd keys per core
HALF = 512
SCALE = 0.125      # 1/sqrt(64)

F32 = mybir.dt.float32
BF16 = mybir.dt.bfloat16
FP16 = mybir.dt.float16
FP8 = mybir.dt.float8e4

# ---- custom DVE op: exp(u/8) ~= (1 + c1 u + c2 u^2 + c3 u^3)^4 ----
# Fitted (Lawson minimax) on |u/8| <= 1.6; max rel err 7.2e-4.
_EC1 = 0.03126080224663743
_EC2 = 0.000493647595612354
_EC3 = 5.0261583805949835e-06


def _register_exp_op():
    from concourse import dve_ops as dops
    from concourse.dve_spec import Spec, Src0, One, C0, C1, C2, sq, lower
    from concourse.dve_uop import DveOpSpec

    name = "EXP4_ANT"
    for op in dops.OPS:
        if op.name == name:
            return op
    body = sq(sq(((C2 * Src0 + C1) * Src0 + C0) * Src0 + One))
    spec = Spec(body=body)
    shas = {}
    for ver in ("v3", "v4"):
        uops = lower(spec, ver=ver)
        shas[ver] = DveOpSpec(name=name, opcode=0, uops=uops, rd1_en=False).sha(ver)
    op = dops.DveOp(name, spec, subdim=False, uops_sha=shas)
    dops.OPS.append(op)
    dops.CUSTOM_DVE_SPECS[name] = spec
    dops._SUB_OPCODE_FOR_NAME[name] = dops._CUSTOM_DVE_ROW_BASE + len(dops.OPS) - 1
    assert max(dops._SUB_OPCODE_FOR_NAME.values()) < 0x20
    return op


def _build():
    """Build + compile the per-core Bass program (SPMD: same NEFF, 8 cores)."""
    exp_op = _register_exp_op()

    nc = bacc.Bacc("TRN2", target_bir_lowering=False, debug=False)

    xT_d = nc.dram_tensor("xT", [E, SK], FP16, kind="ExternalInput")
    W_d = {
        n: nc.dram_tensor(n, [128, E // 128, E], FP16, kind="ExternalInput")
        for n in ("Wq", "Wk", "Wv", "Wo")
    }
    bq_d = nc.dram_tensor("bq", [E], F32, kind="ExternalInput")
    bk_d = nc.dram_tensor("bk", [E], F32, kind="ExternalInput")
    bo_d = nc.dram_tensor("bo_eff", [E], F32, kind="ExternalInput")
    mask_d = nc.dram_tensor("mask8", [128, H, SK // 128], FP16, kind="ExternalInput")
    yT_d = nc.dram_tensor("yT", [E, SQ], F32, kind="ExternalOutput")

    KT = 4   # E // 128 contraction tiles
    NKT = SK // 128  # 16 key tiles

    with tile.TileContext(nc) as tc:
        with (
            nc.allow_low_precision(reason="fp16/fp8 attention kernel"),
            tc.tile_pool(name="singles", bufs=1) as singles,
            tc.tile_pool(name="exps0", bufs=8) as exps0,
            tc.tile_pool(name="exps1", bufs=8) as exps1,
            tc.tile_pool(name="recips", bufs=2) as recips,
            tc.tile_pool(name="avus", bufs=2) as avus,
            tc.tile_pool(name="dscratch", bufs=2, space="DRAM") as dscratch,
            tc.tile_pool(name="bcs", bufs=2) as bcs,
            tc.tile_pool(name="ystage", bufs=3) as ystage,
        ):
            # ---- load everything ----
            W_sb = {}
            for n, d in W_d.items():
                W_sb[n] = singles.tile([128, KT, E], FP16, tag=f"w_{n}", name=f"w_{n}")
            xT_sb = singles.tile([128, KT, SK], FP16)
            # spread the startup-critical loads (x and Wq) round-robin over
            # three DMA queues so the first q-projection starts early
            qs = (nc.sync, nc.scalar, nc.gpsimd)
            for ke in range(KT):
                qs[ke % 3].dma_start(out=xT_sb[:, ke, :], in_=xT_d[ts(ke, 128), :])
                qs[(ke + 1) % 3].dma_start(
                    out=W_sb["Wq"][:, ke, :], in_=W_d["Wq"].ap()[:, ke, :]
                )
            for n, eng in (("Wk", nc.gpsimd), ("Wv", nc.sync), ("Wo", nc.scalar)):
                eng.dma_start(out=W_sb[n], in_=W_d[n].ap())
            bq_sb = singles.tile([128, KT], F32, tag="bq")
            nc.sync.dma_start(out=bq_sb, in_=bq_d.ap().rearrange("(t p) -> p t", p=128))
            bk_sb = singles.tile([128, KT], F32, tag="bk")
            nc.sync.dma_start(out=bk_sb, in_=bk_d.ap().rearrange("(t p) -> p t", p=128))
            bo_sb = singles.tile([128, KT], F32, tag="bo")
            nc.sync.dma_start(out=bo_sb, in_=bo_d.ap().rearrange("(t p) -> p t", p=128))

            # v with ones column (from mask: 0 for padded keys), head-major
            v_sb = singles.tile([128, H, NKT, DH + 1], FP16, tag="v")
            nc.sync.dma_start(out=v_sb[:, :, :, DH], in_=mask_d.ap())

            qT_sb = singles.tile([128, KT, SQ], FP16, tag="qT")
            kT_sb = singles.tile([128, KT, SK], FP16, tag="kT")
            outT_sb = singles.tile([128, KT, SQ], FP16, tag="outT")

            # ---- q/k/v projections ----
            # bias adds on vector, v evacuation on scalar (both idle here;
            # gpsimd cannot read PSUM).
            with tc.tile_pool(name="pproj", bufs=4, space="PSUM") as pproj:
                for th in range(KT):
                    for qc in range(2):
                        ps = pproj.tile([128, 512], F32, tag="pp")
                        for ke in range(KT):
                            nc.tensor.matmul(
                                ps,
                                W_sb["Wq"][:, ke, ts(th, 128)],
                                xT_sb[:, ke, HALF + qc * 512:HALF + (qc + 1) * 512],
                                start=(ke == 0), stop=(ke == KT - 1),
                            )
                        nc.vector.tensor_scalar_add(
                            out=qT_sb[:, th, ts(qc, 512)], in0=ps, scalar1=bq_sb[:, th:th + 1]
                        )
                for th in range(KT):
                    for kc in range(4):
                        ps = pproj.tile([128, 512], F32, tag="pp")
                        for ke in range(KT):
                            nc.tensor.matmul(
                                ps,
                                W_sb["Wk"][:, ke, ts(th, 128)],
                                xT_sb[:, ke, ts(kc, 512)],
                                start=(ke == 0), stop=(ke == KT - 1),
                            )
                        nc.vector.tensor_scalar_add(
                            out=kT_sb[:, th, ts(kc, 512)], in0=ps, scalar1=bk_sb[:, th:th + 1]
                        )
                for st in range(NKT):
                    ps = pproj.tile([128, 512], F32, tag="pp")
                    for ke in range(KT):
                        nc.tensor.matmul(
                            ps,
                            xT_sb[:, ke, ts(st, 128)],
                            W_sb["Wv"][:, ke, :],
                            start=(ke == 0), stop=(ke == KT - 1),
                        )
                    nc.scalar.activation(
                        out=v_sb[:, :, st, 0:DH],
                        in_=ps.rearrange("p (h d) -> p h d", h=H),
                        func=mybir.ActivationFunctionType.Copy,
                    )

            # ---- windowed attention ----
            # Per head-pair hp (heads 2hp on q/k rows 0:64, 2hp+1 on 64:128),
            # per query-half qc (512 queries), stream the 16 key tiles:
            # scores (two concurrent 64-row PE-tile matmuls) -> exp (vector
            # for head 0 of pair, scalar for head 1) into fp8 pair-tiles ->
            # fp8 DoubleRow attn*v accumulation every second key tile.
            # PSUM: 4 score bufs + 4 av accumulators = 8 banks.
            with (
                tc.tile_pool(name="pscore", bufs=6, space="PSUM") as pscore,
                tc.tile_pool(name="pav", bufs=2, space="PSUM") as pav,
            ):
                for hp in range(H // 2):
                    th = hp
                    for qc in range(2):
                        avps = {}
                        # software-pipelined 1 deep: issue scores(kt+1) before
                        # av(kt) so the av weight-loads hide under the score
                        # stream and the exps get a full iteration of slack
                        def scores_exp(kt):
                            s_tiles = []
                            for i in range(2):
                                r0 = 64 * i
                                s_ps = pscore.tile([128, 512], F32, tag="s", name=f"s{i}")
                                nc.tensor.matmul(
                                    s_ps,
                                    kT_sb[r0:r0 + 64, th, ts(kt, 128)],
                                    qT_sb[r0:r0 + 64, th, ts(qc, 512)],
                                    start=True, stop=True,
                                )
                                s_tiles.append(s_ps)
                            e0 = exps0.tile([128, 512], FP16, tag="e0", name="e0")
                            e1 = exps1.tile([128, 512], FP16, tag="e1", name="e1")
                            # custom DVE cubic^4 exp (coefficients fold in SCALE)
                            nc.vector._custom_dve(
                                exp_op, out=e0, in0=s_tiles[0],
                                s0=_EC1, s1=_EC2, imm2=_EC3,
                            )
                            nc.scalar.activation(
                                out=e1, in_=s_tiles[1],
                                func=mybir.ActivationFunctionType.Exp, scale=SCALE,
                            )
                            return e0, e1

                        def av(kt, es):
                            for i, e in enumerate(es):
                                h = 2 * hp + i
                                nc.tensor.matmul(
                                    avps[(qc, i)],
                                    v_sb[:, h, kt, :],
                                    e,
                                    start=(kt == 0), stop=(kt == NKT - 1),
                                )

                        # batch 2 key-tiles of scores, then their 4 av
                        # matmuls back-to-back: halves the PE row-config
                        # switches (64-row score tiles vs 128-row av, ~110ns
                        # each way); larger batches outrun the exp engines
                        esA = scores_exp(0)
                        esB = scores_exp(1)
                        # allocate the av accumulators only now: the pool
                        # wait (on the previous qc's evacuation) then lands
                        # after the first score batch in the tensor stream
                        for i in range(2):
                            avps[(qc, i)] = pav.tile(
                                [DH + 1, 512], F32, tag="av", name=f"av{qc}{i}"
                            )
                        for ktp in range(1, NKT // 2):
                            esC = scores_exp(2 * ktp)
                            esD = scores_exp(2 * ktp + 1)
                            av(2 * ktp - 2, esA)
                            av(2 * ktp - 1, esB)
                            esA, esB = esC, esD
                        av(NKT - 2, esA)
                        av(NKT - 1, esB)

                        # ---- normalize this qc ----
                        # evacuate AV PSUM on vector/scalar (gpsimd can't
                        # read PSUM); the dens row (65th) rides along. Dens
                        # rows gather via SBUF-SBUF DMA, vector computes
                        # 1/dens, a DRAM-roundtrip DMA broadcasts it across
                        # partitions, and gpsimd applies the scale (all-SBUF).
                        avu = avus.tile([DH + 1, 2, 512], F32, tag="avu")
                        nc.vector.tensor_copy(out=avu[:, 0, :], in_=avps[(qc, 0)])
                        nc.scalar.activation(
                            out=avu[:, 1, :], in_=avps[(qc, 1)],
                            func=mybir.ActivationFunctionType.Copy,
                        )
                        dens = recips.tile([2, 512], F32, tag="dens")
                        for i in range(2):
                            nc.sync.dma_start(
                                out=dens[i:i + 1, :], in_=avu[DH:DH + 1, i, :]
                            )
                        recip_f = recips.tile([2, 512], F32, tag="rf")
                        nc.vector.reciprocal_approx_fast(out=recip_f, in_=dens)
                        r_dram = dscratch.tile([2, 512], F32, tag="rd")
                        nc.sync.dma_start(out=r_dram, in_=recip_f)
                        bc_sb = bcs.tile([DH, 2, 512], F32, tag="bc")
                        for i in range(2):
                            nc.sync.dma_start(
                                out=bc_sb[:, i, :],
                                in_=bass.AP(
                                    tensor=r_dram.tensor,
                                    offset=r_dram.offset + i * 512,
                                    ap=[[0, DH]] + [list(a) for a in r_dram.ap[1:]],
                                ),
                            )
                        for i in range(2):
                            r0 = 64 * i
                            # the last chain gates the y projection: vector
                            # (idle by then) beats gpsimd there
                            eng = (nc.vector if (hp == H // 2 - 1 and qc == 1)
                                   else nc.gpsimd)
                            eng.tensor_mul(
                                out=outT_sb[r0:r0 + 64, th, ts(qc, 512)],
                                in0=avu[0:DH, i, :],
                                in1=bc_sb[:, i, :],
                            )

                # ---- output projection ----
                # all 8 output tiles accumulate ke=0..2 first (their outT
                # slices are long done), overlapping the last head-pair's
                # normalization chain; only the ke=3 matmuls wait on it.
                # 6 PSUM tiles come from the score pool, 2 from the av pool.
                mq = [(m, qc) for m in range(KT) for qc in range(2)]
                pss = []
                for g, (m, qc) in enumerate(mq):
                    if g < 6:
                        ps = pscore.tile([128, 512], F32, tag="s", name="yps")
                    else:
                        ps = pav.tile([128, 512], F32, tag="av", name="ypsb")
                    pss.append(ps)
                    for ke in range(KT - 1):
                        nc.tensor.matmul(
                            ps,
                            W_sb["Wo"][:, ke, ts(m, 128)],
                            outT_sb[:, ke, ts(qc, 512)],
                            start=(ke == 0), stop=False,
                        )
                for ps, (m, qc) in zip(pss, mq):
                    nc.tensor.matmul(
                        ps,
                        W_sb["Wo"][:, KT - 1, ts(m, 128)],
                        outT_sb[:, KT - 1, ts(qc, 512)],
                        start=False, stop=True,
                    )
                    yst = ystage.tile([128, 512], F32, tag="y")
                    nc.vector.tensor_scalar_add(out=yst, in0=ps, scalar1=bo_sb[:, m:m + 1])
                    nc.sync.dma_start(out=yT_d[ts(m, 128), ts(qc, 512)], in_=yst)

    nc.compile()
    return nc


_NC_CACHE = []


def _get_nc():
    if not _NC_CACHE:
        _NC_CACHE.append(_build())
    return _NC_CACHE[0]


def _prep_inputs(x, Wq, bq, Wk, bk, Wv, bv, Wo, bo):
    x = np.asarray(x, np.float32)
    xT_full = np.ascontiguousarray(x[0].T)  # [E, S]
    bo_eff = (np.asarray(bo, np.float64)
              + np.asarray(bv, np.float64) @ np.asarray(Wo, np.float64)).astype(np.float32)
    def wprep(W):
        Wb = np.asarray(W, np.float32).astype(np.float16)
        return np.ascontiguousarray(Wb.reshape(4, 128, E).transpose(1, 0, 2))

    shared = {
        "Wq": wprep(Wq),
        "Wk": wprep(Wk),
        "Wv": wprep(Wv),
        "Wo": wprep(Wo),
        "bq": np.asarray(bq, np.float32),
        "bk": np.asarray(bk, np.float32),
        "bo_eff": bo_eff,
    }
    in_maps = []
    for c in range(NCORES):
        g0 = 1024 * c - HALF
        xT_halo = np.zeros((E, SK), np.float32)
        lo, hi = max(0, g0), min(S, g0 + SK)
        xT_halo[:, lo - g0:hi - g0] = xT_full[:, lo:hi]
        mask = np.zeros((SK, H), np.float32)
        mask[lo - g0:hi - g0, :] = 1.0
        mask = np.ascontiguousarray(mask.reshape(SK // 128, 128, H).transpose(1, 2, 0))
        m = dict(shared)
        m["xT"] = xT_halo.astype(np.float16)
        m["mask8"] = mask.astype(np.float16)
        in_maps.append(m)
    return in_maps


def run(inputs: dict, trace: bool = False):
    nc = _get_nc()
    in_maps = _prep_inputs(**inputs)
    res = bass_utils.run_bass_kernel_spmd(
        nc, in_maps, core_ids=list(range(NCORES)), trace=trace
    )
    y = np.concatenate([r["yT"].T for r in res.results], axis=0)[None]
    return np.ascontiguousarray(y.astype(np.float32)), res


def kernel(**inputs) -> np.ndarray:
    y, _ = run(inputs, trace=False)
    return y


# revision 69
# speedup vs baseline: 1.0147x; 1.0036x over previous
"""Sliding-window (chunked) multi-head attention for Trainium2, 8-core SPMD.

Problem: B=1, S=8192, E=512, H=8 heads, Dh=64, window=1024 (half=512).
Reference math per window i (size 1024): keys span [i-512, i+1536).

Sharding: core c owns query window [1024c, 1024c+1024); it receives
x^T for the halo'd key range [1024c-512, 1024c+1536) (zero-padded at
the sequence edges) and computes q/k/v projections locally, windowed
softmax(q k^T / 8) v, and the output projection.  All compute layouts
are transposed ([E, seq]) so every matmul contracts over partitions;
the softmax denominator comes from a ones-augmented v (65th column).
bv is folded into an adjusted output-projection bias on the host
(attn rows sum to 1), so v needs no bias add on-chip.

Pipeline layout: per head-pair and query-half, the attention loop is
software-pipelined one batch deep -- two key-tiles of scores issue
back-to-back (the two heads' 64-row matmuls run as concurrent PE
tiles), then the previous batch's four attn*v matmuls run as a block.
Batching this way halves the PE row-config switches (64-row score
tiles vs 128-row av), which the trace showed cost ~120ns per switch.
exp runs on vector (custom cubic^4 DVE op) for head 0 of the pair and
on the scalar ACT engine for head 1, fully hidden under the tensor
stream.  Normalization is per query-half: vector/scalar evacuate the
AV PSUM (the ones-column denominator rides along), the dens rows
gather via SBUF-SBUF DMA to partition 0 (custom DVE ops misread
nonzero partition offsets), vector computes 1/dens, a DRAM-roundtrip
DMA broadcasts it across 64 partitions, and gpsimd applies the scale.
The y projection accumulates ke=0..2 for all 8 output tiles during
the last normalization chain; only the ke=3 matmuls wait on it.

Outputs are y^T shards [512, 1024] per core; the host transposes and
concatenates.
"""

import numpy as np
import ml_dtypes

import concourse.bass as bass
import concourse.tile as tile
from concourse import bacc, mybir
from concourse import bass_utils
from concourse.bass import ts

# ---- problem constants (hardcoded per contract) ----
S = 8192
E = 512
H = 8
DH = 64
NCORES = 8
SQ = 1024          # queries per core
SK = 2048          # halo'd keys per core
HALF = 512
SCALE = 0.125      # 1/sqrt(64)

F32 = mybir.dt.float32
BF16 = mybir.dt.bfloat16
FP16 = mybir.dt.float16
FP8 = mybir.dt.float8e4

# ---- custom DVE op: exp(u/8) ~= (1 + c1 u + c2 u^2 + c3 u^3)^4 ----
# Fitted (Lawson minimax) on |u/8| <= 1.6; max rel err 7.2e-4.
_EC1 = 0.03126080224663743
_EC2 = 0.000493647595612354
_EC3 = 5.0261583805949835e-06


def _register_exp_op():
    from concourse import dve_ops as dops
    from concourse.dve_spec import Spec, Src0, One, C0, C1, C2, sq, lower
    from concourse.dve_uop import DveOpSpec

    name = "EXP4_ANT"
    for op in dops.OPS:
        if op.name == name:
            return op
    body = sq(sq(((C2 * Src0 + C1) * Src0 + C0) * Src0 + One))
    spec = Spec(body=body)
    shas = {}
    for ver in ("v3", "v4"):
        uops = lower(spec, ver=ver)
        shas[ver] = DveOpSpec(name=name, opcode=0, uops=uops, rd1_en=False).sha(ver)
    op = dops.DveOp(name, spec, subdim=False, uops_sha=shas)
    dops.OPS.append(op)
    dops.CUSTOM_DVE_SPECS[name] = spec
    dops._SUB_OPCODE_FOR_NAME[name] = dops._CUSTOM_DVE_ROW_BASE + len(dops.OPS) - 1
    assert max(dops._SUB_OPCODE_FOR_NAME.values()) < 0x20
    return op


def _build():
    """Build + compile the per-core Bass program (SPMD: same NEFF, 8 cores)."""
    exp_op = _register_exp_op()

    nc = bacc.Bacc("TRN2", target_bir_lowering=False, debug=False)

    xT_d = nc.dram_tensor("xT", [E, SK], FP16, kind="ExternalInput")
    W_d = {
        n: nc.dram_tensor(n, [128, E // 128, E], FP16, kind="ExternalInput")
        for n in ("Wq", "Wk", "Wv", "Wo")
    }
    bq_d = nc.dram_tensor("bq", [E], F32, kind="ExternalInput")
    bk_d = nc.dram_tensor("bk", [E], F32, kind="ExternalInput")
    bo_d = nc.dram_tensor("bo_eff", [E], F32, kind="ExternalInput")
    mask_d = nc.dram_tensor("mask8", [128, H, SK // 128], FP16, kind="ExternalInput")
    yT_d = nc.dram_tensor("yT", [E, SQ], F32, kind="ExternalOutput")

    KT = 4   # E // 128 contraction tiles
    NKT = SK // 128  # 16 key tiles

    with tile.TileContext(nc) as tc:
        with (
            nc.allow_low_precision(reason="fp16/fp8 attention kernel"),
            tc.tile_pool(name="singles", bufs=1) as singles,
            tc.tile_pool(name="exps0", bufs=8) as exps0,
            tc.tile_pool(name="exps1", bufs=8) as exps1,
            tc.tile_pool(name="recips", bufs=2) as recips,
            tc.tile_pool(name="avus", bufs=2) as avus,
            tc.tile_pool(name="dscratch", bufs=2, space="DRAM") as dscratch,
            tc.tile_pool(name="bcs", bufs=2) as bcs,
            tc.tile_pool(name="ystage", bufs=3) as ystage,
        ):
            # ---- load everything ----
            W_sb = {}
            for n, d in W_d.items():
                W_sb[n] = singles.tile([128, KT, E], FP16, tag=f"w_{n}", name=f"w_{n}")
            xT_sb = singles.tile([128, KT, SK], FP16)
            # spread the startup-critical loads (x and Wq) round-robin over
            # three DMA queues so the first q-projection starts early
            qs = (nc.sync, nc.scalar, nc.gpsimd)
            for ke in range(KT):
                qs[ke % 3].dma_start(out=xT_sb[:, ke, :], in_=xT_d[ts(ke, 128), :])
                qs[(ke + 1) % 3].dma_start(
                    out=W_sb["Wq"][:, ke, :], in_=W_d["Wq"].ap()[:, ke, :]
                )
            for n, eng in (("Wk", nc.gpsimd), ("Wv", nc.sync), ("Wo", nc.scalar)):
                eng.dma_start(out=W_sb[n], in_=W_d[n].ap())
            bq_sb = singles.tile([128, KT], F32, tag="bq")
            nc.sync.dma_start(out=bq_sb, in_=bq_d.ap().rearrange("(t p) -> p t", p=128))
            bk_sb = singles.tile([128, KT], F32, tag="bk")
            nc.sync.dma_start(out=bk_sb, in_=bk_d.ap().rearrange("(t p) -> p t", p=128))
            bo_sb = singles.tile([128, KT], F32, tag="bo")
            nc.sync.dma_start(out=bo_sb, in_=bo_d.ap().rearrange("(t p) -> p t", p=128))

            # v with ones column (from mask: 0 for padded keys), head-major
            v_sb = singles.tile([128, H, NKT, DH + 1], FP16, tag="v")
            nc.sync.dma_start(out=v_sb[:, :, :, DH], in_=mask_d.ap())

            qT_sb = singles.tile([128, KT, SQ], FP16, tag="qT")
            kT_sb = singles.tile([128, KT, SK], FP16, tag="kT")
            outT_sb = singles.tile([128, KT, SQ], FP16, tag="outT")

            # ---- q/k/v projections ----
            # bias adds on vector, v evacuation on scalar (both idle here;
            # gpsimd cannot read PSUM).
            with tc.tile_pool(name="pproj", bufs=4, space="PSUM") as pproj:
                for th in range(KT):
                    for qc in range(2):
                        ps = pproj.tile([128, 512], F32, tag="pp")
                        for ke in range(KT):
                            nc.tensor.matmul(
                                ps,
                                W_sb["Wq"][:, ke, ts(th, 128)],
                                xT_sb[:, ke, HALF + qc * 512:HALF + (qc + 1) * 512],
                                start=(ke == 0), stop=(ke == KT - 1),
                            )
                        nc.vector.tensor_scalar_add(
                            out=qT_sb[:, th, ts(qc, 512)], in0=ps, scalar1=bq_sb[:, th:th + 1]
                        )
                for th in range(KT):
                    for kc in range(4):
                        ps = pproj.tile([128, 512], F32, tag="pp")
                        for ke in range(KT):
                            nc.tensor.matmul(
                                ps,
                                W_sb["Wk"][:, ke, ts(th, 128)],
                                xT_sb[:, ke, ts(kc, 512)],
                                start=(ke == 0), stop=(ke == KT - 1),
                            )
                        nc.vector.tensor_scalar_add(
                            out=kT_sb[:, th, ts(kc, 512)], in0=ps, scalar1=bk_sb[:, th:th + 1]
                        )
                for st in range(NKT):
                    ps = pproj.tile([128, 512], F32, tag="pp")
                    for ke in range(KT):
                        nc.tensor.matmul(
                            ps,
                            xT_sb[:, ke, ts(st, 128)],
                            W_sb["Wv"][:, ke, :],
                            start=(ke == 0), stop=(ke == KT - 1),
                        )
                    nc.scalar.activation(
                        out=v_sb[:, :, st, 0:DH],
                        in_=ps.rearrange("p (h d) -> p h d", h=H),
                        func=mybir.ActivationFunctionType.Copy,
                    )

            # ---- windowed attention ----
            # Per head-pair hp (heads 2hp on q/k rows 0:64, 2hp+1 on 64:128),
            # per query-half qc (512 queries), stream the 16 key tiles:
            # scores (two concurrent 64-row PE-tile matmuls) -> exp (vector
            # for head 0 of pair, scalar for head 1) into fp8 pair-tiles ->
            # fp8 DoubleRow attn*v accumulation every second key tile.
            # PSUM: 4 score bufs + 4 av accumulators = 8 banks.
            with (
                tc.tile_pool(name="pscore", bufs=6, space="PSUM") as pscore,
                tc.tile_pool(name="pav", bufs=2, space="PSUM") as pav,
            ):
                for hp in range(H // 2):
                    th = hp
                    for qc in range(2):
                        avps = {}
                        # software-pipelined 1 deep: issue scores(kt+1) before
                        # av(kt) so the av weight-loads hide under the score
                        # stream and the exps get a full iteration of slack
                        def scores_exp(kt):
                            s_tiles = []
                            for i in range(2):
                                r0 = 64 * i
                                s_ps = pscore.tile([128, 512], F32, tag="s", name=f"s{i}")
                                nc.tensor.matmul(
                                    s_ps,
                                    kT_sb[r0:r0 + 64, th, ts(kt, 128)],
                                    qT_sb[r0:r0 + 64, th, ts(qc, 512)],
                                    start=True, stop=True,
                                )
                                s_tiles.append(s_ps)
                            e0 = exps0.tile([128, 512], FP16, tag="e0", name="e0")
                            e1 = exps1.tile([128, 512], FP16, tag="e1", name="e1")
                            # custom DVE cubic^4 exp (coefficients fold in SCALE)
                            nc.vector._custom_dve(
                                exp_op, out=e0, in0=s_tiles[0],
                                s0=_EC1, s1=_EC2, imm2=_EC3,
                            )
                            nc.scalar.activation(
                                out=e1, in_=s_tiles[1],
                                func=mybir.ActivationFunctionType.Exp, scale=SCALE,
                            )
                            return e0, e1

                        def av(kt, es):
                            for i, e in enumerate(es):
                                h = 2 * hp + i
                                nc.tensor.matmul(
                                    avps[(qc, i)],
                                    v_sb[:, h, kt, :],
                                    e,
                                    start=(kt == 0), stop=(kt == NKT - 1),
                                )

                        # batch 2 key-tiles of scores, then their 4 av
                        # matmuls back-to-back: halves the PE row-config
                        # switches (64-row score tiles vs 128-row av, ~110ns
                        # each way); larger batches outrun the exp engines
                        esA = scores_exp(0)
                        esB = scores_exp(1)
                        # allocate the av accumulators only now: the pool
                        # wait (on the previous qc's evacuation) then lands
                        # after the first score batch in the tensor stream
                        for i in range(2):
                            avps[(qc, i)] = pav.tile(
                                [DH + 1, 512], F32, tag="av", name=f"av{qc}{i}"
                            )
                        for ktp in range(1, NKT // 2):
                            esC = scores_exp(2 * ktp)
                            esD = scores_exp(2 * ktp + 1)
                            av(2 * ktp - 2, esA)
                            av(2 * ktp - 1, esB)
                            esA, esB = esC, esD
                        av(NKT - 2, esA)
                        av(NKT - 1, esB)

                        # ---- normalize this qc ----
                        # evacuate AV PSUM on vector/scalar (gpsimd can't
                        # read PSUM); the dens row (65th) rides along. Dens
                        # rows gather via SBUF-SBUF DMA, vector computes
                        # 1/dens, a DRAM-roundtrip DMA broadcasts it across
                        # partitions, and gpsimd applies the scale (all-SBUF).
                        avu = avus.tile([DH + 1, 2, 512], F32, tag="avu")
                        nc.vector.tensor_copy(out=avu[:, 0, :], in_=avps[(qc, 0)])
                        nc.scalar.activation(
                            out=avu[:, 1, :], in_=avps[(qc, 1)],
                            func=mybir.ActivationFunctionType.Copy,
                        )
                        dens = recips.tile([2, 512], F32, tag="dens")
                        for i in range(2):
                            nc.sync.dma_start(
                                out=dens[i:i + 1, :], in_=avu[DH:DH + 1, i, :]
                            )
                        recip_f = recips.tile([2, 512], F32, tag="rf")
                        nc.vector.reciprocal_approx_fast(out=recip_f, in_=dens)
                        r_dram = dscratch.tile([2, 512], F32, tag="rd")
                        nc.sync.dma_start(out=r_dram, in_=recip_f)
                        bc_sb = bcs.tile([DH, 2, 512], F32, tag="bc")
                        for i in range(2):
                            nc.sync.dma_start(
                                out=bc_sb[:, i, :],
                                in_=bass.AP(
                                    tensor=r_dram.tensor,
                                    offset=r_dram.offset + i * 512,
                                    ap=[[0, DH]] + [list(a) for a in r_dram.ap[1:]],
                                ),
                            )
                        for i in range(2):
                            r0 = 64 * i
                            # the last chain gates the y projection: vector
                            # (idle by then) beats gpsimd there
                            eng = (nc.vector if (hp == H // 2 - 1 and qc == 1)
                                   else nc.gpsimd)
                            eng.tensor_mul(
                                out=outT_sb[r0:r0 + 64, th, ts(qc, 512)],
                                in0=avu[0:DH, i, :],
                                in1=bc_sb[:, i, :],
                            )

                # ---- output projection ----
                # all 8 output tiles accumulate ke=0..2 first (their outT
                # slices are long done), overlapping the last head-pair's
                # normalization chain; only the ke=3 matmuls wait on it.
                # 6 PSUM tiles come from the score pool, 2 from the av pool.
                mq = [(m, qc) for m in range(KT) for qc in range(2)]
                pss = []
                for g, (m, qc) in enumerate(mq):
                    if g < 6:
                        ps = pscore.tile([128, 512], F32, tag="s", name="yps")
                    else:
                        ps = pav.tile([128, 512], F32, tag="av", name="ypsb")
                    pss.append(ps)
                    for ke in range(KT - 1):
                        nc.tensor.matmul(
                            ps,
                            W_sb["Wo"][:, ke, ts(m, 128)],
                            outT_sb[:, ke, ts(qc, 512)],
                            start=(ke == 0), stop=False,
                        )
                for g, (ps, (m, qc)) in enumerate(zip(pss, mq)):
                    nc.tensor.matmul(
                        ps,
                        W_sb["Wo"][:, KT - 1, ts(m, 128)],
                        outT_sb[:, KT - 1, ts(qc, 512)],
                        start=False, stop=True,
                    )
                    yst = ystage.tile([128, 512], F32, tag="y")
                    # split the bias-add evacuations across vector and scalar
                    # (Identity activation takes a per-partition AP bias) and
                    # the stores across three DMA queues: the final drain is
                    # serial otherwise
                    if g % 2 == 0:
                        nc.vector.tensor_scalar_add(
                            out=yst, in0=ps, scalar1=bo_sb[:, m:m + 1]
                        )
                    else:
                        nc.scalar.activation(
                            out=yst, in_=ps,
                            func=mybir.ActivationFunctionType.Identity,
                            bias=bo_sb[:, m:m + 1],
                        )
                    eng = (nc.sync, nc.scalar, nc.gpsimd)[g % 3]
                    eng.dma_start(out=yT_d[ts(m, 128), ts(qc, 512)], in_=yst)

    nc.compile()
    return nc


_NC_CACHE = []


def _get_nc():
    if not _NC_CACHE:
        _NC_CACHE.append(_build())
    return _NC_CACHE[0]


def _prep_inputs(x, Wq, bq, Wk, bk, Wv, bv, Wo, bo):
    x = np.asarray(x, np.float32)
    xT_full = np.ascontiguousarray(x[0].T)  # [E, S]
    bo_eff = (np.asarray(bo, np.float64)
              + np.asarray(bv, np.float64) @ np.asarray(Wo, np.float64)).astype(np.float32)
    def wprep(W):
        Wb = np.asarray(W, np.float32).astype(np.float16)
        return np.ascontiguousarray(Wb.reshape(4, 128, E).transpose(1, 0, 2))

    shared = {
        "Wq": wprep(Wq),
        "Wk": wprep(Wk),
        "Wv": wprep(Wv),
        "Wo": wprep(Wo),
        "bq": np.asarray(bq, np.float32),
        "bk": np.asarray(bk, np.float32),
        "bo_eff": bo_eff,
    }
    in_maps = []
    for c in range(NCORES):
        g0 = 1024 * c - HALF
        xT_halo = np.zeros((E, SK), np.float32)
        lo, hi = max(0, g0), min(S, g0 + SK)
        xT_halo[:, lo - g0:hi - g0] = xT_full[:, lo:hi]
        mask = np.zeros((SK, H), np.float32)
        mask[lo - g0:hi - g0, :] = 1.0
        mask = np.ascontiguousarray(mask.reshape(SK // 128, 128, H).transpose(1, 2, 0))
        m = dict(shared)
        m["xT"] = xT_halo.astype(np.float16)
        m["mask8"] = mask.astype(np.float16)
        in_maps.append(m)
    return in_maps


def run(inputs: dict, trace: bool = False):
    nc = _get_nc()
    in_maps = _prep_inputs(**inputs)
    res = bass_utils.run_bass_kernel_spmd(
        nc, in_maps, core_ids=list(range(NCORES)), trace=trace
    )
    y = np.concatenate([r["yT"].T for r in res.results], axis=0)[None]
    return np.ascontiguousarray(y.astype(np.float32)), res


def kernel(**inputs) -> np.ndarray:
    y, _ = run(inputs, trace=False)
    return y
